# revision 1
# baseline (speedup 1.0000x reference)
"""GroupGAT kernel for Trainium2 (Bass/Tile), 8-core data-parallel.

Math restructure (attention weights commute with @W):
    H = h @ W;  e[b,n] = lrelu(H_self.a1 + H[b,n].a2)
              = lrelu(h_self.(W a1) + h[b,n].(W a2))       <- dots in h-space
    ws = softmax(e) @ H = (softmax(e) @ h) @ W             <- weighted sum in h-space
    out = elu(H_ally[:,0] + ws_ally + ws_opp)
        = elu((h_self + hw_ally) @ W_ally + hw_opp @ W_opp)
so the big per-node matmuls collapse into per-node dot products (DVE) and
two [128,128] matmuls per batch tile (PE).
"""

import numpy as np

import concourse.bass as bass
import concourse.bacc as bacc
import concourse.mybir as mybir
from concourse import tile
from concourse.bass_utils import run_bass_kernel_spmd

N_CORES = 8
B = 16384
NUM_NODE = 41
NA = 20  # num_ally
NO = 20  # num_opp
D = 128
B_SHARD = B // N_CORES
P = 128  # partitions / batch tile
NEG_INF = -1e9

F32 = mybir.dt.float32
AL = mybir.AluOpType
AF = mybir.ActivationFunctionType


def build_nc(b_shard=B_SHARD, repeats=1):
    n_tiles = b_shard // P
    nc = bacc.Bacc("TRN2", target_bir_lowering=False, debug=False)

    h_d = nc.dram_tensor("h", [b_shard, NUM_NODE, D], F32, kind="ExternalInput").ap()
    mneg_d = nc.dram_tensor("mneg", [b_shard, 42], F32, kind="ExternalInput").ap()
    vrep_d = nc.dram_tensor("vrep", [P, 4 * D], F32, kind="ExternalInput").ap()
    wcat_d = nc.dram_tensor("wcat", [P, 2 * D], F32, kind="ExternalInput").ap()
    ident_d = nc.dram_tensor("ident", [P, P], F32, kind="ExternalInput").ap()
    out_d = nc.dram_tensor("out", [b_shard, D], F32, kind="ExternalOutput").ap()

    with tile.TileContext(nc) as tc:
        with (
            tc.tile_pool(name="const", bufs=1) as cpool,
            tc.tile_pool(name="hin", bufs=3) as hpool,
            tc.tile_pool(name="small", bufs=3) as spool,
            tc.tile_pool(name="work", bufs=3) as wpool,
            tc.tile_pool(name="psum", bufs=2, space=bass.MemorySpace.PSUM) as ppool,
        ):
            vrep = cpool.tile([P, 4 * D], F32)
            wcat = cpool.tile([P, 2 * D], F32)
            ident = cpool.tile([P, P], F32)
            nc.sync.dma_start(vrep[:], vrep_d[:])
            nc.sync.dma_start(wcat[:], wcat_d[:])
            nc.sync.dma_start(ident[:], ident_d[:])
            v1a, v2a = vrep[:, 0:D], vrep[:, D : 2 * D]
            v1o, v2o = vrep[:, 2 * D : 3 * D], vrep[:, 3 * D : 4 * D]
            w_ally, w_opp = wcat[:, 0:D], wcat[:, D : 2 * D]

            import contextlib

            rep_ctx = (
                tc.For_i(0, repeats, 1) if repeats > 1 else contextlib.nullcontext()
            )
            with rep_ctx:
                _body(nc, tc, n_tiles, h_d, mneg_d, out_d, hpool, spool, wpool,
                      ppool, v1a, v2a, v1o, v2o, w_ally, w_opp, ident)

    nc.compile()
    return nc


def _body(nc, tc, n_tiles, h_d, mneg_d, out_d, hpool, spool, wpool, ppool,
          v1a, v2a, v1o, v2o, w_ally, w_opp, ident):
    if True:  # keep indentation of the original loop body
            for it in range(n_tiles):
                b0 = it * P
                h_t = hpool.tile([P, NUM_NODE, D], F32)
                mneg_t = spool.tile([P, 42], F32)
                nc.sync.dma_start(h_t[:], h_d[b0 : b0 + P])
                nc.sync.dma_start(mneg_t[:], mneg_d[b0 : b0 + P])

                prod = wpool.tile([P, D], F32, tag="prod")
                s1 = spool.tile([P, 2], F32)
                e_a = spool.tile([P, 21], F32)
                e_o = spool.tile([P, 21], F32)

                # --- dots: s1x = h_self . v1x ; e_x[:,n] = h[n] . v2x  (+ s1x below)
                # (tensor_tensor_reduce faults the DVE on HW; scalar_tensor_tensor
                # with accum_out is the working multiply-reduce form)
                nc.vector.scalar_tensor_tensor(
                    prod[:], h_t[:, 0, :], 1.0, v1a, AL.mult, AL.mult, accum_out=s1[:, 0:1]
                )
                prod_o = wpool.tile([P, D], F32, tag="prod_o")
                nc.vector.scalar_tensor_tensor(
                    prod_o[:], h_t[:, 0, :], 1.0, v1o, AL.mult, AL.mult, accum_out=s1[:, 1:2]
                )
                for n in range(21):
                    nc.vector.scalar_tensor_tensor(
                        prod[:], h_t[:, n, :], 1.0, v2a,
                        AL.mult, AL.mult, accum_out=e_a[:, n : n + 1],
                    )
                for n in range(21):
                    src = 0 if n == 0 else NA + n
                    nc.vector.scalar_tensor_tensor(
                        prod_o[:], h_t[:, src, :], 1.0, v2o,
                        AL.mult, AL.mult, accum_out=e_o[:, n : n + 1],
                    )
                nc.vector.tensor_scalar_add(e_a[:], e_a[:], s1[:, 0:1])
                nc.vector.tensor_scalar_add(e_o[:], e_o[:], s1[:, 1:2])

                # --- leaky relu: lrelu(x) = max(0.2*x, x); then additive mask
                nc.vector.scalar_tensor_tensor(
                    e_a[:], e_a[:], 0.2, e_a[:], AL.mult, AL.max
                )
                nc.vector.scalar_tensor_tensor(
                    e_o[:], e_o[:], 0.2, e_o[:], AL.mult, AL.max
                )
                nc.vector.tensor_add(e_a[:], e_a[:], mneg_t[:, 0:21])
                nc.vector.tensor_add(e_o[:], e_o[:], mneg_t[:, 21:42])

                # --- exp (no max-sub needed; |e| <~ 20) + fused denominator
                expe_a = spool.tile([P, 21], F32)
                expe_o = spool.tile([P, 21], F32)
                den = spool.tile([P, 2], F32)
                rec = spool.tile([P, 2], F32)
                nc.scalar.activation(expe_a[:], e_a[:], AF.Exp, accum_out=den[:, 0:1])
                nc.scalar.activation(expe_o[:], e_o[:], AF.Exp, accum_out=den[:, 1:2])
                nc.vector.reciprocal(rec[:], den[:])

                # --- unnormalized weighted sums hw = sum_n expe[:,n] * h[:,n,:]
                # ally group via PE: diag(expe_n) @ h_n accumulated in PSUM
                # (ACT builds diag_n = ident * expe[:,n] with per-partition scale)
                diag_t = wpool.tile([P, 21, D], F32, tag="diag")
                hwps_a = ppool.tile([P, D], F32)
                for n in range(21):
                    nc.scalar.activation(
                        diag_t[:, n, :], ident[:], AF.Copy,
                        scale=expe_a[:, n : n + 1],
                    )
                for n in range(21):
                    nc.tensor.matmul(
                        hwps_a[:], diag_t[:, n, :], h_t[:, n, :],
                        start=(n == 0), stop=(n == 20),
                    )
                hw_o = wpool.tile([P, D], F32)
                nc.vector.tensor_scalar_mul(hw_o[:], h_t[:, 0, :], expe_o[:, 0:1])
                for n in range(1, 21):
                    nc.vector.scalar_tensor_tensor(
                        hw_o[:], h_t[:, NA + n, :], expe_o[:, n : n + 1], hw_o[:],
                        AL.mult, AL.add,
                    )

                # --- x_a = h_self + hw_a/den_a ; x_o = hw_o/den_o
                x_a = wpool.tile([P, D], F32)
                x_o = wpool.tile([P, D], F32)
                nc.vector.scalar_tensor_tensor(
                    x_a[:], hwps_a[:], rec[:, 0:1], h_t[:, 0, :], AL.mult, AL.add
                )
                nc.vector.tensor_scalar_mul(x_o[:], hw_o[:], rec[:, 1:2])

                # --- out = elu(x_a @ W_ally + x_o @ W_opp)
                tr_a = ppool.tile([P, D], F32)
                tr_o = ppool.tile([P, D], F32)
                xT_a = wpool.tile([P, D], F32)
                xT_o = wpool.tile([P, D], F32)
                nc.tensor.transpose(tr_a[:], x_a[:], ident[:])
                nc.tensor.transpose(tr_o[:], x_o[:], ident[:])
                nc.scalar.copy(xT_a[:], tr_a[:])
                nc.scalar.copy(xT_o[:], tr_o[:])
                mm = ppool.tile([P, D], F32)
                nc.tensor.matmul(mm[:], xT_a[:], w_ally, start=True, stop=False)
                nc.tensor.matmul(mm[:], xT_o[:], w_opp, start=False, stop=True)

                # elu(x) = max(x, exp(min(x,0)) - 1)
                t1 = wpool.tile([P, D], F32)
                out_t = wpool.tile([P, D], F32)
                nc.vector.tensor_scalar_min(t1[:], mm[:], 0.0)
                nc.scalar.activation(t1[:], t1[:], AF.Exp)
                nc.vector.scalar_tensor_tensor(
                    out_t[:], t1[:], -1.0, mm[:], AL.add, AL.max
                )
                nc.sync.dma_start(out_d[b0 : b0 + P], out_t[:])


_NC_CACHE = {}


def _get_nc(b_shard):
    if b_shard not in _NC_CACHE:
        _NC_CACHE[b_shard] = build_nc(b_shard)
    return _NC_CACHE[b_shard]


def _host_precompute(h, W_ally, W_opp, a_ally, a_opp, mask):
    b = h.shape[0]
    v1a = (W_ally @ a_ally[:D, 0]).astype(np.float32)
    v2a = (W_ally @ a_ally[D:, 0]).astype(np.float32)
    v1o = (W_opp @ a_opp[:D, 0]).astype(np.float32)
    v2o = (W_opp @ a_opp[D:, 0]).astype(np.float32)
    vrep = np.concatenate(
        [np.broadcast_to(v[None, :], (P, D)) for v in (v1a, v2a, v1o, v2o)], axis=1
    ).astype(np.float32)
    vrep = np.ascontiguousarray(vrep)
    wcat = np.ascontiguousarray(
        np.concatenate([W_ally, W_opp], axis=1).astype(np.float32)
    )
    ident = np.eye(P, dtype=np.float32)
    mneg = np.zeros((b, 42), np.float32)
    mneg[:, 1:21] = np.where(mask[:, 1 : 1 + NA], NEG_INF, 0.0)
    mneg[:, 22:42] = np.where(mask[:, 1 + NA :], NEG_INF, 0.0)
    return vrep, wcat, ident, mneg


def kernel(h, W_ally, W_opp, a_ally, a_opp, mask, num_ally, num_opp):
    assert int(num_ally) == NA and int(num_opp) == NO
    h = np.ascontiguousarray(np.asarray(h, dtype=np.float32))
    mask = np.asarray(mask)
    W_ally = np.asarray(W_ally, dtype=np.float32)
    W_opp = np.asarray(W_opp, dtype=np.float32)
    a_ally = np.asarray(a_ally, dtype=np.float32)
    a_opp = np.asarray(a_opp, dtype=np.float32)

    vrep, wcat, ident, mneg = _host_precompute(h, W_ally, W_opp, a_ally, a_opp, mask)

    nc = _get_nc(B_SHARD)
    in_maps = []
    for c in range(N_CORES):
        s = slice(c * B_SHARD, (c + 1) * B_SHARD)
        in_maps.append(
            {
                "h": h[s],
                "mneg": np.ascontiguousarray(mneg[s]),
                "vrep": vrep,
                "wcat": wcat,
                "ident": ident,
            }
        )
    res = run_bass_kernel_spmd(nc, in_maps, core_ids=list(range(N_CORES)))
    global LAST_RESULTS
    LAST_RESULTS = res
    return np.concatenate([res.results[c]["out"] for c in range(N_CORES)], axis=0)


LAST_RESULTS = None

